# revision 1
# baseline (speedup 1.0000x reference)
"""Multi-head attention (B=2, N=2048, d_model=1024, 16 heads x 64) on 8
Trainium2 NeuronCores.

Sharding: batch x head-group. Core c handles batch b = c//4 and heads
4*(c%4) .. 4*(c%4)+3. Projection weights are column-sliced (rows for Wo) so
each core computes q/k/v projections only for its 4 heads, full attention
for those heads, and a partial output projection. The host sums the four
partial outputs per batch (tensor-parallel reduce on to_out) and adds bo.

Device kernel (per core), matmuls in fp32r (rne-11 mantissa):
  qT/kT : projections producing [head-dim, seq] (lhsT = W chunk)
  v     : natural [seq, head-dim] with a ones column folded in (M=65)
  ST    : k^T q per head -> scores^T [keys, queries]; K=64 row-tile PAIRS
          (two heads concurrently on PE tiles T0/T8)
  E     : exp(ST * scale) via ScalarE eviction PSUM->SBUF (the wall:
          ~1 elem/lane/cycle regardless of dtype)
  AV    : [v|ones]^T @ E -> [65, 512]: rows 0-63 = O^T, row 64 = denom
  norm  : one batched reciprocal per query block, denom broadcast via a
          K=4 pattern matmul, DVE multiply into O^T
  out   : O^T-as-lhsT @ Wo slice -> partial [2048, 1024]
"""

import numpy as np

import concourse.mybir as mybir
import concourse.tile as tile
from concourse import bacc
from concourse import bass_utils
from concourse.tile_rust import add_dep_helper

F32 = mybir.dt.float32
F32R = mybir.dt.float32r
EXP = mybir.ActivationFunctionType.Exp

B = 2
N = 2048
D_MODEL = 1024
NHEAD = 16
DIM_HEAD = 64
SCALE = DIM_HEAD ** (-0.5)
N_CORES = 8
HEADS_PER_CORE = 4          # 2 pairs
INNER = HEADS_PER_CORE * DIM_HEAD  # 256

QB = 512                    # query block
N_QB = N // QB              # 4
N_KC = N // 128             # 16 key chunks


def _rne11(x: np.ndarray) -> np.ndarray:
    """Round fp32 to fp32r (round-to-nearest-even, 11 mantissa bits)."""
    b = np.ascontiguousarray(x, dtype=np.float32).view(np.uint32)
    lsb = (b >> np.uint32(12)) & np.uint32(1)
    r = (b + np.uint32(0x7FF) + lsb) & np.uint32(0xFFFFF000)
    return r.view(np.float32)


def build_nc():
    nc = bacc.Bacc("TRN2", target_bir_lowering=False, debug=False,
                   num_devices=N_CORES)
    xqt = nc.dram_tensor("xqt", [D_MODEL, N], F32R, kind="ExternalInput").ap()
    xkt = nc.dram_tensor("xkt", [D_MODEL, N], F32R, kind="ExternalInput").ap()
    xvt = nc.dram_tensor("xvt", [D_MODEL, N], F32R, kind="ExternalInput").ap()
    wq = nc.dram_tensor("wq", [D_MODEL, INNER], F32R, kind="ExternalInput").ap()
    wk = nc.dram_tensor("wk", [D_MODEL, INNER], F32R, kind="ExternalInput").ap()
    wv = nc.dram_tensor("wv", [D_MODEL, INNER], F32R, kind="ExternalInput").ap()
    wo = nc.dram_tensor("wo", [INNER, D_MODEL], F32R, kind="ExternalInput").ap()
    vones = nc.dram_tensor("vones", [128, N_KC, HEADS_PER_CORE, 1], F32R,
                           kind="ExternalInput").ap()
    # bc pattern: pat4[k, p, m] = 1 where head k owns output rows m in pair p
    pat4 = nc.dram_tensor("pat4", [128, 2, 128], F32R, kind="ExternalInput").ap()
    out = nc.dram_tensor("out", [N, D_MODEL], F32, kind="ExternalOutput").ap()

    with tile.TileContext(nc) as tc:
        with (
            tc.tile_pool(name="wpool", bufs=1) as wpool,
            tc.tile_pool(name="persist", bufs=1) as persist,
            tc.tile_pool(name="xin", bufs=3) as xin,
            tc.tile_pool(name="ering", bufs=9) as ering,
            tc.tile_pool(name="stage", bufs=3) as stage,
            tc.tile_pool(name="ps_st", bufs=2, space="PSUM") as ps_st,
            tc.tile_pool(name="ps_av", bufs=1, space="PSUM") as ps_av,
            tc.tile_pool(name="ps_misc", bufs=2, space="PSUM") as ps_misc,
        ):
            # ---- weights on sync queue, ordered by first use ----
            wk_sb = wpool.tile([128, 8, INNER], F32R)
            nc.sync.dma_start(wk_sb[:], wk.rearrange("(c p) m -> p c m", p=128))
            wq_sb = wpool.tile([128, 8, INNER], F32R)
            nc.sync.dma_start(wq_sb[:], wq.rearrange("(c p) m -> p c m", p=128))

            qt_sb = persist.tile([128, 2, N], F32R)
            kt_sb = persist.tile([128, 2, N], F32R)
            v_sb = persist.tile([128, N_KC, HEADS_PER_CORE, DIM_HEAD + 1], F32R)
            ot_sb = persist.tile([128, 2, N], F32R)

            xqt_r = xqt.rearrange("(c p) n -> p c n", p=128)
            xkt_r = xkt.rearrange("(c p) n -> p c n", p=128)
            xvt_r = xvt.rearrange("(c p) n -> p c n", p=128)

            def emit_kt(n):
                ns = slice(n * QB, (n + 1) * QB)
                xk_t = xin.tile([128, 8, QB], F32R, tag="xin", name=f"xk_{n}")
                nc.scalar.dma_start(xk_t[:], xkt_r[:, :, ns])
                for m in range(2):
                    pk = ps_misc.tile([128, QB], F32, tag="mp", name=f"pk{n}{m}")
                    for c in range(8):
                        nc.tensor.matmul(
                            pk[:], wk_sb[:, c, m * 128:(m + 1) * 128],
                            xk_t[:, c, :], start=(c == 0), stop=(c == 7))
                    nc.vector.tensor_copy(kt_sb[:, m, ns], pk[:])

            def emit_qt(n):
                ns = slice(n * QB, (n + 1) * QB)
                xq_t = xin.tile([128, 8, QB], F32R, tag="xin", name=f"xq_{n}")
                nc.sync.dma_start(xq_t[:], xqt_r[:, :, ns])
                for m in range(2):
                    pq = ps_misc.tile([128, QB], F32, tag="mp", name=f"pq{n}{m}")
                    for c in range(8):
                        nc.tensor.matmul(
                            pq[:], wq_sb[:, c, m * 128:(m + 1) * 128],
                            xq_t[:, c, :], start=(c == 0), stop=(c == 7))
                    nc.vector.tensor_copy(qt_sb[:, m, ns], pq[:])

            def emit_vblock(n):
                ns = slice(n * QB, (n + 1) * QB)
                xv_t = xin.tile([128, 8, QB], F32R, tag="xin", name=f"xv_{n}")
                nc.scalar.dma_start(xv_t[:], xvt_r[:, :, ns])
                for kci in range(4):
                    kc = n * 4 + kci
                    kcs = slice(kci * 128, (kci + 1) * 128)
                    pva = ps_misc.tile([128, INNER], F32, tag="mp",
                                       padded_shape=[128, 512], name=f"pva{kc}")
                    pvb = ps_misc.tile([128, INNER], F32, tag="mp",
                                       padded_shape=[128, 512], name=f"pvb{kc}")
                    for c in range(8):
                        nc.tensor.matmul(
                            pva[:], xv_t[0:64, c, kcs],
                            wv_sb[0:64, c, :], start=(c == 0), stop=(c == 7))
                        nc.tensor.matmul(
                            pvb[:], xv_t[64:128, c, kcs],
                            wv_sb[64:128, c, :], start=(c == 0), stop=(c == 7))
                    va_sb = stage.tile([128, INNER], F32, tag="vasb",
                                       name=f"va{kc}", bufs=2)
                    nc.vector.tensor_copy(va_sb[:], pva[:])
                    nc.vector.scalar_tensor_tensor(
                        v_sb[:, kc, :, 0:DIM_HEAD],
                        pvb[:].rearrange("p (h d) -> p h d", h=HEADS_PER_CORE),
                        1.0,
                        va_sb[:].rearrange("p (h d) -> p h d", h=HEADS_PER_CORE),
                        mybir.AluOpType.mult, mybir.AluOpType.add)

            def emit_outproj_chunk(qb, idx):
                qc = qb * 4 + idx // 2
                dc = idx % 2
                cs = slice(qc * 128, (qc + 1) * 128)
                op = ps_misc.tile([128, 512], F32, tag="mp", name=f"op{qc}{dc}")
                for ic in range(2):
                    nc.tensor.matmul(
                        op[:], ot_sb[:, ic, cs],
                        wo_sb[:, ic, dc * 512:(dc + 1) * 512],
                        start=(ic == 0), stop=(ic == 1))
                o_stage = stage.tile([128, 512], F32, tag="ostage",
                                     name=f"ost{qc}{dc}", bufs=2)
                nc.vector.tensor_copy(o_stage[:], op[:])
                nc.sync.dma_start(out[cs, dc * 512:(dc + 1) * 512], o_stage[:])

            qb_state = {}

            def emit_st(qb, p, kc):
                qs = slice(qb * QB, (qb + 1) * QB)
                ks = slice(kc * 128, (kc + 1) * 128)
                st = ps_st.tile([128, 1024], F32, tag="st", name=f"st{qb}{p}{kc}")
                mm0 = nc.tensor.matmul(st[:, 0:512], kt_sb[0:64, p, ks],
                                       qt_sb[0:64, p, qs], start=True, stop=True)
                nc.tensor.matmul(st[:, 512:1024], kt_sb[64:128, p, ks],
                                 qt_sb[64:128, p, qs], start=True, stop=True)
                e_t = ering.tile([128, 2, 512], F32R, tag="e",
                                 name=f"e{qb}{p}{kc}")
                nc.scalar.activation(
                    e_t[:], st[:].rearrange("p (h n) -> p h n", h=2),
                    EXP, scale=float(SCALE))
                return e_t, mm0

            def emit_av(qb, p, kc, avs, e_t):
                for hh in range(2):
                    nc.tensor.matmul(
                        avs[hh][0:DIM_HEAD + 1, :],
                        v_sb[:, kc, 2 * p + hh, :], e_t[:, hh, :],
                        start=(kc == 0), stop=(kc == N_KC - 1))

            def evict_pair(qb, p, avs):
                den4 = qb_state[qb]["den4"]
                avsb = []
                for hh in range(2):
                    a_sb = stage.tile([DIM_HEAD + 1, 512], F32, tag="avsb",
                                      name=f"avsb{qb}_{p}_{hh}", bufs=4)
                    nc.vector.tensor_copy(a_sb[:], avs[hh][0:DIM_HEAD + 1, :])
                    k32 = 32 * (2 * p + hh)
                    nc.vector.tensor_copy(den4[k32:k32 + 1, :],
                                          a_sb[DIM_HEAD:DIM_HEAD + 1, :])
                    avsb.append(a_sb)
                return avsb

            def finalize_pair(qb, p, avsb, order_after=None):
                den4 = qb_state[qb]["den4"]
                qs = slice(qb * QB, (qb + 1) * QB)
                rec = stage.tile([128, 512], F32, tag="rec",
                                 name=f"rec{qb}{p}", bufs=2)
                with nc.allow_low_precision(reason="softmax denom recip"):
                    nc.vector.reciprocal(rec[:], den4[:])
                recr = stage.tile([128, 512], F32R, tag="recr",
                                  name=f"recr{qb}{p}", bufs=2)
                nc.vector.tensor_copy(recr[:], rec[:])
                bc = ps_misc.tile([128, 512], F32, tag="mp", name=f"bc{qb}{p}")
                bcmm = nc.tensor.matmul(bc[:], pat_sb[:, p, :], recr[:],
                                        start=True, stop=True)
                if order_after is not None:
                    add_dep_helper(order_after.ins, bcmm.ins, sync=False,
                                   reason="hold bc behind ST stream")
                for hh in range(2):
                    nc.vector.tensor_mul(
                        ot_sb[hh * 64:(hh + 1) * 64, p, qs],
                        avsb[hh][0:DIM_HEAD, :],
                        bc[hh * 64:(hh + 1) * 64, :])


            def begin_qb(qb):
                den4 = stage.tile([128, 512], F32, tag="den4", name=f"den{qb}",
                                  bufs=1)
                nc.vector.memset(den4[:], 1.0)
                qb_state[qb] = dict(den4=den4)

            def new_avs(qb, p):
                return [ps_av.tile([128, 512], F32, tag=f"av{hh}",
                                   name=f"av{hh}_{qb}_{p}")
                        for hh in range(2)]

            def phase_fillers(qb, p):
                f = []
                if qb == 0 and p == 0:
                    for n in range(1, N_QB):
                        f.append((4 * n - 1, lambda n=n: (emit_kt(n),
                                                          emit_vblock(n))))
                elif qb == 0 and p == 1:
                    f.append((7, lambda: emit_qt(1)))
                else:
                    prev = qb - 1
                    if p == 0:
                        for g in range(4):
                            f.append(((9, 11, 13, 15)[g],
                                      lambda g=g: emit_outproj_chunk(prev, g)))
                    else:
                        if qb < N_QB - 1:
                            f.append((5, lambda: emit_qt(qb + 1)))
                        for g in range(4):
                            f.append(((3, 7, 10, 13)[g],
                                      lambda g=g: emit_outproj_chunk(prev, 4 + g)))
                return dict(f)

            emit_kt(0)
            emit_qt(0)

            wv_sb = wpool.tile([128, 8, INNER], F32R)
            nc.sync.dma_start(wv_sb[:], wv.rearrange("(c p) m -> p c m", p=128))
            nc.sync.dma_start(v_sb[:, :, :, DIM_HEAD:DIM_HEAD + 1], vones[:])
            wo_sb = wpool.tile([128, 2, D_MODEL], F32R)
            nc.sync.dma_start(wo_sb[:], wo.rearrange("(c p) d -> p c d", p=128))
            pat_sb = wpool.tile([128, 2, 128], F32R)
            nc.sync.dma_start(pat_sb[:], pat4[:])

            emit_vblock(0)

            AV_LAG = 4
            phases = [(qb, p) for qb in range(N_QB) for p in range(2)]
            pending = None      # (qb, p, avs, [(kc, e_t)...])
            pending_fin = None  # (qb, p, avsb)

            for qb, p in phases:
                if p == 0:
                    begin_qb(qb)
                avs = new_avs(qb, p)
                fillers = phase_fillers(qb, p)
                eq = []
                for kc in range(N_KC):
                    e_t, stmm = emit_st(qb, p, kc)
                    eq.append((kc, e_t))
                    if kc == AV_LAG - 1 and pending is not None:
                        pq, pp, pavs, peq = pending
                        for pkc, pe_t in peq:
                            emit_av(pq, pp, pkc, pavs, pe_t)
                        pending_fin = (pq, pp, evict_pair(pq, pp, pavs))
                        pending = None
                    if kc == 7 and pending_fin is not None:
                        fq, fp, favsb = pending_fin
                        finalize_pair(fq, fp, favsb, order_after=stmm)
                        pending_fin = None
                    if kc >= AV_LAG:
                        pkc, pe_t = eq[kc - AV_LAG]
                        emit_av(qb, p, pkc, avs, pe_t)
                    if kc in fillers:
                        fillers[kc]()
                pending = (qb, p, avs, eq[N_KC - AV_LAG:])

            pq, pp, pavs, peq = pending
            for pkc, pe_t in peq:
                emit_av(pq, pp, pkc, pavs, pe_t)
            finalize_pair(pq, pp, evict_pair(pq, pp, pavs))
            for idx in range(8):
                emit_outproj_chunk(N_QB - 1, idx)
    nc.compile()
    return nc


_NC_CACHE = None


def _get_nc():
    global _NC_CACHE
    if _NC_CACHE is None:
        _NC_CACHE = build_nc()
    return _NC_CACHE


def _make_pat4():
    pat = np.zeros((128, 2, 128), np.float32)
    for p in range(2):
        for hh in range(2):
            pat[32 * (2 * p + hh), p, hh * 64:(hh + 1) * 64] = 1.0
    return pat


def make_in_maps(query, key, value, Wq, Wk, Wv, Wo):
    query = np.asarray(query, np.float32)
    key = np.asarray(key, np.float32)
    value = np.asarray(value, np.float32)
    vones = np.ones((128, N_KC, HEADS_PER_CORE, 1), np.float32)
    pat4 = _make_pat4()
    in_maps = []
    for c in range(N_CORES):
        b = c // 4
        hg = c % 4
        cols = slice(hg * INNER, (hg + 1) * INNER)
        in_maps.append({
            "xqt": _rne11(np.asarray(query[b]).T),
            "xkt": _rne11(np.asarray(key[b]).T),
            "xvt": _rne11(np.asarray(value[b]).T),
            "wq": _rne11(np.asarray(Wq[:, cols])),
            "wk": _rne11(np.asarray(Wk[:, cols])),
            "wv": _rne11(np.asarray(Wv[:, cols])),
            "wo": _rne11(np.asarray(Wo[cols, :])),
            "vones": vones,
            "pat4": pat4,
        })
    return in_maps


def kernel(query, key, value, Wq, Wk, Wv, Wo, bo, _trace=False, _trace_cores=None):
    nc = _get_nc()
    in_maps = make_in_maps(query, key, value, Wq, Wk, Wv, Wo)
    res = bass_utils.run_bass_kernel_spmd(
        nc, in_maps, core_ids=list(range(N_CORES)), trace=_trace,
        trace_cores=_trace_cores)
    out = np.zeros((B, N, D_MODEL), np.float32)
    for c in range(N_CORES):
        out[c // 4] += res.results[c]["out"]
    out += np.asarray(bo, np.float32)[None, None, :]
    if _trace:
        return out, res
    return out



# revision 8
# speedup vs baseline: 1.0548x; 1.0548x over previous
"""Multi-head attention (B=2, N=2048, d_model=1024, 16 heads x 64) on 8
Trainium2 NeuronCores.

Sharding: batch x head-group. Core c handles batch b = c//4 and heads
4*(c%4) .. 4*(c%4)+3. Projection weights are column-sliced (rows for Wo) so
each core computes q/k/v projections only for its 4 heads, full attention
for those heads, and a partial output projection. The host sums the four
partial outputs per batch (tensor-parallel reduce on to_out) and adds bo.

v2: bf16 end-to-end on SBUF/DRAM (halves DMA + DVE + SBUF, enables FWL
weight loads); scores drain to bf16 PSUM so one ACTIVATE evicts two
key-chunks (2048 elem/partition) cutting ScalarE exp time; fast
reciprocal for the softmax denominator; input streams spread across
engine DMA queues.

Device kernel (per core):
  qT/kT : projections producing [head-dim, seq] (lhsT = W chunk)
  v     : natural [seq, head-dim] with a ones column (M=65) for the
          softmax denominator
  ST    : k^T q per head -> scores^T [keys, queries]; two heads run
          concurrently on PE row tiles (K=64 each), drain to bf16 PSUM
  E     : exp(ST * scale) via one ScalarE activation per double-chunk
          (2 kc x 2 heads = 2048 elem/partition), PSUM->SBUF bf16
  AV    : [v|ones]^T @ E -> [65, 512] fp32 PSUM accumulated over kc
  norm  : reciprocal of denom row, broadcast via a pattern matmul,
          DVE multiply into O^T
  out   : O^T-as-lhsT @ Wo slice -> partial [2048, 1024] bf16
"""

import numpy as np
import ml_dtypes

import concourse.mybir as mybir
import concourse.tile as tile
from concourse import bacc
from concourse import bass_utils
from concourse.tile_rust import add_dep_helper

F32 = mybir.dt.float32
BF16 = mybir.dt.bfloat16
EXP = mybir.ActivationFunctionType.Exp
NP_BF16 = ml_dtypes.bfloat16

B = 2
N = 2048
D_MODEL = 1024
NHEAD = 16
DIM_HEAD = 64
SCALE = DIM_HEAD ** (-0.5)
N_CORES = 8
HEADS_PER_CORE = 4          # 2 pairs
INNER = HEADS_PER_CORE * DIM_HEAD  # 256

QB = 512                    # query block
N_QB = N // QB              # 4
N_KC = N // 128             # 16 key chunks
N_DC = N_KC // 2            # 8 double-chunks per phase


def build_nc():
    nc = bacc.Bacc("TRN2", target_bir_lowering=False, debug=False,
                   num_devices=N_CORES)
    xqt = nc.dram_tensor("xqt", [D_MODEL, N], BF16, kind="ExternalInput").ap()
    xkt = nc.dram_tensor("xkt", [D_MODEL, N], BF16, kind="ExternalInput").ap()
    xvt = nc.dram_tensor("xvt", [D_MODEL, N], BF16, kind="ExternalInput").ap()
    wq = nc.dram_tensor("wq", [D_MODEL, INNER], BF16, kind="ExternalInput").ap()
    wk = nc.dram_tensor("wk", [D_MODEL, INNER], BF16, kind="ExternalInput").ap()
    wv = nc.dram_tensor("wv", [D_MODEL, INNER], BF16, kind="ExternalInput").ap()
    wo = nc.dram_tensor("wo", [INNER, D_MODEL], BF16, kind="ExternalInput").ap()
    # bc pattern: pat4[k, p, m] = 1 where head k owns output rows m in pair p
    pat4 = nc.dram_tensor("pat4", [128, 2, 128], BF16, kind="ExternalInput").ap()
    out = nc.dram_tensor("out", [N, D_MODEL], BF16, kind="ExternalOutput").ap()

    with tile.TileContext(nc) as tc:
        with (
            tc.tile_pool(name="wpool", bufs=1) as wpool,
            tc.tile_pool(name="persist", bufs=1) as persist,
            tc.tile_pool(name="xin", bufs=4) as xin,
            tc.tile_pool(name="ering", bufs=9) as ering,
            tc.tile_pool(name="stage", bufs=3) as stage,
            tc.tile_pool(name="ps_st", bufs=2, space="PSUM") as ps_st,
            tc.tile_pool(name="ps_av", bufs=1, space="PSUM") as ps_av,
            tc.tile_pool(name="ps_misc", bufs=2, space="PSUM") as ps_misc,
        ):
            # ---- weights, ordered by first use; x streams on other queues ----
            wk_sb = wpool.tile([128, 8, INNER], BF16)
            nc.sync.dma_start(wk_sb[:], wk.rearrange("(c p) m -> p c m", p=128))
            wq_sb = wpool.tile([128, 8, INNER], BF16)
            nc.sync.dma_start(wq_sb[:], wq.rearrange("(c p) m -> p c m", p=128))

            qt_sb = persist.tile([128, 2, N], BF16)
            kt_sb = persist.tile([128, 2, N], BF16)
            v_sb = persist.tile([128, N_KC, HEADS_PER_CORE, DIM_HEAD + 1], BF16)
            ot_sb = persist.tile([128, 2, N], BF16)

            xqt_r = xqt.rearrange("(c p) n -> p c n", p=128)
            xkt_r = xkt.rearrange("(c p) n -> p c n", p=128)
            xvt_r = xvt.rearrange("(c p) n -> p c n", p=128)

            def emit_kt(n):
                ns = slice(n * QB, (n + 1) * QB)
                xk_t = xin.tile([128, 8, QB], BF16, tag="xin", name=f"xk_{n}")
                nc.scalar.dma_start(xk_t[:], xkt_r[:, :, ns])
                for m in range(2):
                    pk = ps_misc.tile([128, QB], F32, tag="mp", name=f"pk{n}{m}")
                    for c in range(8):
                        nc.tensor.matmul(
                            pk[:], wk_sb[:, c, m * 128:(m + 1) * 128],
                            xk_t[:, c, :], start=(c == 0), stop=(c == 7))
                    nc.vector.tensor_copy(kt_sb[:, m, ns], pk[:])

            def emit_qt(n):
                ns = slice(n * QB, (n + 1) * QB)
                xq_t = xin.tile([128, 8, QB], BF16, tag="xin", name=f"xq_{n}")
                nc.scalar.dma_start(xq_t[:], xqt_r[:, :, ns])
                for m in range(2):
                    pq = ps_misc.tile([128, QB], F32, tag="mp", name=f"pq{n}{m}")
                    for c in range(8):
                        nc.tensor.matmul(
                            pq[:], wq_sb[:, c, m * 128:(m + 1) * 128],
                            xq_t[:, c, :], start=(c == 0), stop=(c == 7))
                    nc.vector.tensor_copy(qt_sb[:, m, ns], pq[:])

            def emit_vblock(n):
                ns = slice(n * QB, (n + 1) * QB)
                xv_t = xin.tile([128, 8, QB], BF16, tag="xin", name=f"xv_{n}")
                nc.gpsimd.dma_start(xv_t[:], xvt_r[:, :, ns])
                for kci in range(4):
                    kc = n * 4 + kci
                    kcs = slice(kci * 128, (kci + 1) * 128)
                    pva = ps_misc.tile([128, INNER], F32, tag="mp",
                                       padded_shape=[128, 512], name=f"pva{kc}")
                    pvb = ps_misc.tile([128, INNER], F32, tag="mp",
                                       padded_shape=[128, 512], name=f"pvb{kc}")
                    for c in range(8):
                        nc.tensor.matmul(
                            pva[:], xv_t[0:64, c, kcs],
                            wv_sb[0:64, c, :], start=(c == 0), stop=(c == 7))
                        nc.tensor.matmul(
                            pvb[:], xv_t[64:128, c, kcs],
                            wv_sb[64:128, c, :], start=(c == 0), stop=(c == 7))
                    va_sb = stage.tile([128, INNER], F32, tag="vasb",
                                       name=f"va{kc}", bufs=2)
                    nc.vector.tensor_copy(va_sb[:], pva[:])
                    nc.vector.scalar_tensor_tensor(
                        v_sb[:, kc, :, 0:DIM_HEAD],
                        pvb[:].rearrange("p (h d) -> p h d", h=HEADS_PER_CORE),
                        1.0,
                        va_sb[:].rearrange("p (h d) -> p h d", h=HEADS_PER_CORE),
                        mybir.AluOpType.mult, mybir.AluOpType.add)

            def emit_outproj_chunk(qb, idx):
                qc = qb * 4 + idx // 2
                dc = idx % 2
                cs = slice(qc * 128, (qc + 1) * 128)
                op = ps_misc.tile([128, 512], F32, tag="mp", name=f"op{qc}{dc}")
                for ic in range(2):
                    nc.tensor.matmul(
                        op[:], ot_sb[:, ic, cs],
                        wo_sb[:, ic, dc * 512:(dc + 1) * 512],
                        start=(ic == 0), stop=(ic == 1))
                o_stage = stage.tile([128, 512], BF16, tag="ostage",
                                     name=f"ost{qc}{dc}", bufs=2)
                nc.vector.tensor_copy(o_stage[:], op[:])
                nc.sync.dma_start(out[cs, dc * 512:(dc + 1) * 512], o_stage[:])

            qb_state = {}

            def emit_st(qb, p, kc):
                qs = slice(qb * QB, (qb + 1) * QB)
                ks = slice(kc * 128, (kc + 1) * 128)
                st = ps_st.tile([128, 1024], F32, tag="st", name=f"st{qb}{p}{kc}")
                mm0 = nc.tensor.matmul(st[:, 0:512], kt_sb[0:64, p, ks],
                                       qt_sb[0:64, p, qs], start=True, stop=True)
                nc.tensor.matmul(st[:, 512:1024], kt_sb[64:128, p, ks],
                                 qt_sb[64:128, p, qs], start=True, stop=True)
                e_t = ering.tile([128, 2, 512], BF16, tag="e",
                                 name=f"e{qb}{p}{kc}")
                nc.scalar.activation(
                    e_t[:], st[:].rearrange("p (h n) -> p h n", h=2),
                    EXP, scale=float(SCALE))
                return e_t, mm0

            def emit_av(qb, p, kc, avs, e_t):
                for hh in range(2):
                    nc.tensor.matmul(
                        avs[hh][0:DIM_HEAD + 1, :],
                        v_sb[:, kc, 2 * p + hh, :], e_t[:, hh, :],
                        start=(kc == 0), stop=(kc == N_KC - 1))

            def evict_pair(qb, p, avs):
                den4 = qb_state[qb]["den4"]
                avsb = []
                for hh in range(2):
                    a_sb = stage.tile([DIM_HEAD + 1, 512], BF16, tag="avsb",
                                      name=f"avsb{qb}_{p}_{hh}", bufs=4)
                    nc.vector.tensor_copy(a_sb[:], avs[hh][0:DIM_HEAD + 1, :])
                    k32 = 32 * (2 * p + hh)
                    nc.vector.tensor_copy(den4[k32:k32 + 1, :],
                                          avs[hh][DIM_HEAD:DIM_HEAD + 1, :])
                    avsb.append(a_sb)
                return avsb

            def finalize_pair(qb, p, avsb, order_after=None):
                den4 = qb_state[qb]["den4"]
                qs = slice(qb * QB, (qb + 1) * QB)
                recr = stage.tile([128, 512], BF16, tag="recr",
                                  name=f"recr{qb}{p}", bufs=2)
                with nc.allow_low_precision(reason="softmax denom recip"):
                    nc.vector.reciprocal(recr[:], den4[:])
                bc = ps_misc.tile([128, 512], F32, tag="mp", name=f"bc{qb}{p}")
                bcmm = nc.tensor.matmul(bc[:], pat_sb[:, p, :], recr[:],
                                        start=True, stop=True)
                if order_after is not None:
                    add_dep_helper(order_after.ins, bcmm.ins, sync=False,
                                   reason="hold bc behind ST stream")
                for hh in range(2):
                    nc.vector.tensor_mul(
                        ot_sb[hh * 64:(hh + 1) * 64, p, qs],
                        avsb[hh][0:DIM_HEAD, :],
                        bc[hh * 64:(hh + 1) * 64, :])

            def begin_qb(qb):
                den4 = stage.tile([128, 512], F32, tag="den4", name=f"den{qb}",
                                  bufs=1)
                nc.vector.memset(den4[:], 1.0)
                qb_state[qb] = dict(den4=den4)

            def new_avs(qb, p):
                return [ps_av.tile([128, 512], F32, tag=f"av{hh}",
                                   name=f"av{hh}_{qb}_{p}")
                        for hh in range(2)]

            def phase_fillers(qb, p):
                f = {}
                def add(dc, fn):
                    f.setdefault(dc, []).append(fn)
                if qb == 0 and p == 0:
                    for n in range(1, N_QB):
                        add(4 * n - 1, lambda n=n: (emit_kt(n),
                                                    emit_vblock(n)))
                elif qb == 0 and p == 1:
                    add(7, lambda: emit_qt(1))
                else:
                    prev = qb - 1
                    if p == 0:
                        for g in range(4):
                            add((9, 11, 13, 15)[g],
                                lambda g=g: emit_outproj_chunk(prev, g))
                    else:
                        if qb < N_QB - 1:
                            add(5, lambda: emit_qt(qb + 1))
                        for g in range(4):
                            add((3, 7, 10, 13)[g],
                                lambda g=g: emit_outproj_chunk(prev, 4 + g))
                return f

            emit_kt(0)
            emit_qt(0)

            wv_sb = wpool.tile([128, 8, INNER], BF16)
            nc.gpsimd.dma_start(wv_sb[:], wv.rearrange("(c p) m -> p c m", p=128))
            nc.vector.memset(v_sb[:, :, :, DIM_HEAD:DIM_HEAD + 1], 1.0)
            wo_sb = wpool.tile([128, 2, D_MODEL], BF16)
            nc.sync.dma_start(wo_sb[:], wo.rearrange("(c p) d -> p c d", p=128))
            pat_sb = wpool.tile([128, 2, 128], BF16)
            nc.sync.dma_start(pat_sb[:], pat4[:])

            emit_vblock(0)

            AV_LAG = 4
            phases = [(qb, p) for qb in range(N_QB) for p in range(2)]
            pending = None      # (qb, p, avs, [(kc, e_t)...])
            pending_fin = None  # (qb, p, avsb)

            for qb, p in phases:
                if p == 0:
                    begin_qb(qb)
                avs = new_avs(qb, p)
                fillers = phase_fillers(qb, p)
                eq = []
                for kc in range(N_KC):
                    e_t, stmm = emit_st(qb, p, kc)
                    eq.append((kc, e_t))
                    if kc == AV_LAG - 1 and pending is not None:
                        pq, pp, pavs, peq = pending
                        for pkc, pe_t in peq:
                            emit_av(pq, pp, pkc, pavs, pe_t)
                        pending_fin = (pq, pp, evict_pair(pq, pp, pavs))
                        pending = None
                    if kc == 7 and pending_fin is not None:
                        fq, fp, favsb = pending_fin
                        finalize_pair(fq, fp, favsb, order_after=stmm)
                        pending_fin = None
                    if kc >= AV_LAG:
                        pkc, pe_t = eq[kc - AV_LAG]
                        emit_av(qb, p, pkc, avs, pe_t)
                    for fn in fillers.get(kc, ()):
                        fn()
                pending = (qb, p, avs, eq[N_KC - AV_LAG:])

            pq, pp, pavs, peq = pending
            for pkc, pe_t in peq:
                emit_av(pq, pp, pkc, pavs, pe_t)
            finalize_pair(pq, pp, evict_pair(pq, pp, pavs))
            for idx in range(8):
                emit_outproj_chunk(N_QB - 1, idx)
    nc.compile()
    return nc


_NC_CACHE = None


def _get_nc():
    global _NC_CACHE
    if _NC_CACHE is None:
        _NC_CACHE = build_nc()
    return _NC_CACHE


def _make_pat4():
    pat = np.zeros((128, 2, 128), np.float32)
    for p in range(2):
        for hh in range(2):
            pat[32 * (2 * p + hh), p, hh * 64:(hh + 1) * 64] = 1.0
    return pat.astype(NP_BF16)


def make_in_maps(query, key, value, Wq, Wk, Wv, Wo):
    query = np.asarray(query, np.float32)
    key = np.asarray(key, np.float32)
    value = np.asarray(value, np.float32)
    pat4 = _make_pat4()
    in_maps = []
    for c in range(N_CORES):
        b = c // 4
        hg = c % 4
        cols = slice(hg * INNER, (hg + 1) * INNER)
        in_maps.append({
            "xqt": np.ascontiguousarray(query[b].T).astype(NP_BF16),
            "xkt": np.ascontiguousarray(key[b].T).astype(NP_BF16),
            "xvt": np.ascontiguousarray(value[b].T).astype(NP_BF16),
            "wq": np.ascontiguousarray(Wq[:, cols]).astype(NP_BF16),
            "wk": np.ascontiguousarray(Wk[:, cols]).astype(NP_BF16),
            "wv": np.ascontiguousarray(Wv[:, cols]).astype(NP_BF16),
            "wo": np.ascontiguousarray(Wo[cols, :]).astype(NP_BF16),
            "pat4": pat4,
        })
    return in_maps


def kernel(query, key, value, Wq, Wk, Wv, Wo, bo, _trace=False, _trace_cores=None):
    nc = _get_nc()
    in_maps = make_in_maps(query, key, value, Wq, Wk, Wv, Wo)
    res = bass_utils.run_bass_kernel_spmd(
        nc, in_maps, core_ids=list(range(N_CORES)), trace=_trace,
        trace_cores=_trace_cores)
    out = np.zeros((B, N, D_MODEL), np.float32)
    for c in range(N_CORES):
        out[c // 4] += np.asarray(res.results[c]["out"], np.float32)
    out += np.asarray(bo, np.float32)[None, None, :]
    if _trace:
        return out, res
    return out


# revision 18
# speedup vs baseline: 1.1588x; 1.0986x over previous
"""Multi-head attention (B=2, N=2048, d_model=1024, 16 heads x 64) on 8
Trainium2 NeuronCores.

Sharding: batch x head-group. Core c handles batch b = c//4 and heads
4*(c%4) .. 4*(c%4)+3. Projection weights are column-sliced (rows for Wo) so
each core computes q/k/v projections only for its 4 heads, full attention
for those heads, and a partial output projection. The host sums the four
partial outputs per batch (tensor-parallel reduce on to_out) and adds bo.

v3 structure (per core):
  - x / W inputs in bf16 (halves HBM traffic); intermediates in fp32r.
  - warmup matmuls during the initial DMA wait so the PE HAM clock gate
    is released before real work arrives.
  - ST: k^T q per head pair -> scores^T, two heads concurrently on PE
    row tiles; one ScalarE exp eviction per key chunk (the steady-state
    rate limiter at ~1.15us per 128x1024 tile).
  - AV: [v|ones]^T @ E accumulated over key chunks in fp32 PSUM.
  - All projection / output work is chopped into ~2-matmul micro-tasks
    dispatched between ST slots so the scores->exp pipeline never
    starves while PE queues stay dense.
  - softmax denominator: ones column of v, reciprocal_approx_fast,
    pattern-matmul broadcast, DVE multiply into O^T (bf16).
  - out: O^T-as-lhsT @ Wo slice -> partial [2048, 1024] fp32.
"""

import numpy as np
import ml_dtypes

import concourse.mybir as mybir
import concourse.tile as tile
from concourse import bacc
from concourse import bass_utils
from concourse.tile_rust import add_dep_helper

F32 = mybir.dt.float32
F32R = mybir.dt.float32r
BF16 = mybir.dt.bfloat16
EXP = mybir.ActivationFunctionType.Exp
NP_BF16 = ml_dtypes.bfloat16

B = 2
N = 2048
D_MODEL = 1024
NHEAD = 16
DIM_HEAD = 64
SCALE = DIM_HEAD ** (-0.5)
N_CORES = 8
HEADS_PER_CORE = 4          # 2 pairs
INNER = HEADS_PER_CORE * DIM_HEAD  # 256

QB = 512                    # query block
N_QB = N // QB              # 4
N_KC = N // 128             # 16 key chunks


def _rne11(x: np.ndarray) -> np.ndarray:
    """Round fp32 to fp32r (round-to-nearest-even, 11 mantissa bits)."""
    b = np.ascontiguousarray(x, dtype=np.float32).view(np.uint32)
    lsb = (b >> np.uint32(12)) & np.uint32(1)
    r = (b + np.uint32(0x7FF) + lsb) & np.uint32(0xFFFFF000)
    return r.view(np.float32)


def build_nc():
    nc = bacc.Bacc("TRN2", target_bir_lowering=False, debug=False,
                   num_devices=N_CORES)
    xqt = nc.dram_tensor("xqt", [D_MODEL, N], BF16, kind="ExternalInput").ap()
    xkt = nc.dram_tensor("xkt", [D_MODEL, N], BF16, kind="ExternalInput").ap()
    xvt = nc.dram_tensor("xvt", [D_MODEL, N], BF16, kind="ExternalInput").ap()
    wq = nc.dram_tensor("wq", [D_MODEL, INNER], BF16, kind="ExternalInput").ap()
    wk = nc.dram_tensor("wk", [D_MODEL, INNER], BF16, kind="ExternalInput").ap()
    wv = nc.dram_tensor("wv", [D_MODEL, INNER], BF16, kind="ExternalInput").ap()
    wo = nc.dram_tensor("wo", [INNER, D_MODEL], BF16, kind="ExternalInput").ap()
    vones = nc.dram_tensor("vones", [128, N_KC, HEADS_PER_CORE, 1], F32R,
                           kind="ExternalInput").ap()
    # bc pattern: pat4[k, p, m] = 1 where head k owns output rows m in pair p
    pat4 = nc.dram_tensor("pat4", [128, 2, 128], F32R, kind="ExternalInput").ap()
    out = nc.dram_tensor("out", [N, D_MODEL], F32, kind="ExternalOutput").ap()

    with tile.TileContext(nc) as tc:
        with (
            tc.tile_pool(name="wpool", bufs=1) as wpool,
            tc.tile_pool(name="persist", bufs=1) as persist,
            tc.tile_pool(name="xk_p", bufs=2) as xk_p,
            tc.tile_pool(name="xq_p", bufs=2) as xq_p,
            tc.tile_pool(name="xv_p", bufs=2) as xv_p,
            tc.tile_pool(name="ering", bufs=9) as ering,
            tc.tile_pool(name="stage", bufs=3) as stage,
            tc.tile_pool(name="ps_st", bufs=2, space="PSUM") as ps_st,
            tc.tile_pool(name="ps_av", bufs=1, space="PSUM") as ps_av,
            tc.tile_pool(name="ps_misc", bufs=2, space="PSUM") as ps_misc,
        ):
            # ---- weight DMAs: wk/xk chain on sync (HWDGE), the rest on
            # gpsimd (SWDGE) so the scalar queue carries ONLY activations.
            wk_sb = wpool.tile([128, 8, INNER], BF16)
            nc.sync.dma_start(wk_sb[:], wk.rearrange("(c p) m -> p c m", p=128))
            wq_sb = wpool.tile([128, 8, INNER], BF16)
            nc.gpsimd.dma_start(wq_sb[:], wq.rearrange("(c p) m -> p c m", p=128))

            qt_sb = persist.tile([128, 2, N], F32R)
            kt_sb = persist.tile([128, 2, N], F32R)
            v_sb = persist.tile([128, N_KC, HEADS_PER_CORE, DIM_HEAD + 1], F32R)
            ot_sb = persist.tile([128, 2, N], BF16)

            xqt_r = xqt.rearrange("(c p) n -> p c n", p=128)
            xkt_r = xkt.rearrange("(c p) n -> p c n", p=128)
            xvt_r = xvt.rearrange("(c p) n -> p c n", p=128)

            # ---- PE warmup: ~15 tiny matmuls during the initial DMA wait
            # release the HAM clock gate (~3.4us of activity) so real
            # projections run at 2.4 GHz from the start.
            warm_sb = stage.tile([128, 64], F32, tag="warm", name="warm",
                                 bufs=1)
            nc.vector.memset(warm_sb[:], 1.0)
            pwarm = ps_misc.tile([64, 64], F32, tag="mp", name="pwarm",
                                 padded_shape=[128, 512])
            for _ in range(15):
                nc.tensor.matmul(pwarm[:], warm_sb[:, 0:64], warm_sb[:],
                                 start=True, stop=True)

            tasks = []          # FIFO of PE micro-tasks (~<=2 big matmuls)

            xk_tiles = {}
            xq_tiles = {}

            def emit_kt(n, ms=(0, 1)):
                ns = slice(n * QB, (n + 1) * QB)
                if n not in xk_tiles:
                    xk_tiles[n] = xk_p.tile([128, 8, QB], BF16, tag="xk",
                                            name=f"xk_{n}")
                    nc.sync.dma_start(xk_tiles[n][:], xkt_r[:, :, ns])
                xk_t = xk_tiles[n]
                for m in ms:
                    st_ = {}
                    def head(m=m, st_=st_, xk_t=xk_t, n=n):
                        st_['pk'] = ps_misc.tile([128, QB], F32, tag="mp",
                                                 name=f"pk{n}{m}")
                        for c in (0, 1):
                            nc.tensor.matmul(
                                st_['pk'][:], wk_sb[:, c, m * 128:(m + 1) * 128],
                                xk_t[:, c, :], start=(c == 0), stop=False)
                    def mid(cs, m=m, st_=st_, xk_t=xk_t):
                        def f():
                            for c in cs:
                                nc.tensor.matmul(
                                    st_['pk'][:],
                                    wk_sb[:, c, m * 128:(m + 1) * 128],
                                    xk_t[:, c, :], start=False, stop=False)
                        return f
                    def fin(m=m, st_=st_, xk_t=xk_t, ns=ns):
                        for c in (6, 7):
                            nc.tensor.matmul(
                                st_['pk'][:], wk_sb[:, c, m * 128:(m + 1) * 128],
                                xk_t[:, c, :], start=False, stop=(c == 7))
                        nc.vector.tensor_copy(kt_sb[:, m, ns], st_['pk'][:])
                    tasks.extend([head, mid((2, 3)), mid((4, 5)), fin])

            def emit_qt(n, ms=(0, 1)):
                ns = slice(n * QB, (n + 1) * QB)
                if n not in xq_tiles:
                    xq_tiles[n] = xq_p.tile([128, 8, QB], BF16, tag="xq",
                                            name=f"xq_{n}")
                    nc.gpsimd.dma_start(xq_tiles[n][:], xqt_r[:, :, ns])
                xq_t = xq_tiles[n]
                for m in ms:
                    st_ = {}
                    def head(m=m, st_=st_, xq_t=xq_t, n=n):
                        st_['pq'] = ps_misc.tile([128, QB], F32, tag="mp",
                                                 name=f"pq{n}{m}")
                        for c in (0, 1):
                            nc.tensor.matmul(
                                st_['pq'][:], wq_sb[:, c, m * 128:(m + 1) * 128],
                                xq_t[:, c, :], start=(c == 0), stop=False)
                    def mid(cs, m=m, st_=st_, xq_t=xq_t):
                        def f():
                            for c in cs:
                                nc.tensor.matmul(
                                    st_['pq'][:],
                                    wq_sb[:, c, m * 128:(m + 1) * 128],
                                    xq_t[:, c, :], start=False, stop=False)
                        return f
                    def fin(m=m, st_=st_, xq_t=xq_t, ns=ns):
                        for c in (6, 7):
                            nc.tensor.matmul(
                                st_['pq'][:], wq_sb[:, c, m * 128:(m + 1) * 128],
                                xq_t[:, c, :], start=False, stop=(c == 7))
                        nc.vector.tensor_copy(qt_sb[:, m, ns], st_['pq'][:])
                    tasks.extend([head, mid((2, 3)), mid((4, 5)), fin])

            def emit_vblock(n):
                ns = slice(n * QB, (n + 1) * QB)
                xv_t = xv_p.tile([128, 8, QB], BF16, tag="xv", name=f"xv_{n}")
                nc.gpsimd.dma_start(xv_t[:], xvt_r[:, :, ns])
                for kci in range(4):
                    kc = n * 4 + kci
                    kcs = slice(kci * 128, (kci + 1) * 128)
                    st_ = {}
                    def piece(cs, first, last, kc=kc, kcs=kcs, st_=st_,
                              xv_t=xv_t):
                        def f():
                            if first:
                                st_['pva'] = ps_misc.tile(
                                    [128, INNER], F32, tag="mp",
                                    padded_shape=[128, 512], name=f"pva{kc}")
                                st_['pvb'] = ps_misc.tile(
                                    [128, INNER], F32, tag="mp",
                                    padded_shape=[128, 512], name=f"pvb{kc}")
                            for c in cs:
                                nc.tensor.matmul(
                                    st_['pva'][:], xv_t[0:64, c, kcs],
                                    wv_sb[0:64, c, :], start=(c == 0),
                                    stop=(c == 7))
                                nc.tensor.matmul(
                                    st_['pvb'][:], xv_t[64:128, c, kcs],
                                    wv_sb[64:128, c, :], start=(c == 0),
                                    stop=(c == 7))
                            if last:
                                va_sb = stage.tile([128, INNER], F32,
                                                   tag="vasb", name=f"va{kc}",
                                                   bufs=2)
                                nc.vector.tensor_copy(va_sb[:], st_['pva'][:])
                                nc.vector.scalar_tensor_tensor(
                                    v_sb[:, kc, :, 0:DIM_HEAD],
                                    st_['pvb'][:].rearrange(
                                        "p (h d) -> p h d", h=HEADS_PER_CORE),
                                    1.0,
                                    va_sb[:].rearrange(
                                        "p (h d) -> p h d", h=HEADS_PER_CORE),
                                    mybir.AluOpType.mult, mybir.AluOpType.add)
                        return f
                    tasks.append(piece((0, 1, 2, 3), True, False))
                    tasks.append(piece((4, 5, 6, 7), False, True))

            def emit_outproj_chunk(qb, idx, evict_eng="vector"):
                qc = qb * 4 + idx // 2
                dc = idx % 2
                cs = slice(qc * 128, (qc + 1) * 128)
                def f():
                    op = ps_misc.tile([128, 512], F32, tag="mp",
                                      name=f"op{qc}{dc}")
                    for ic in range(2):
                        nc.tensor.matmul(
                            op[:], ot_sb[:, ic, cs],
                            wo_sb[:, ic, dc * 512:(dc + 1) * 512],
                            start=(ic == 0), stop=(ic == 1))
                    o_stage = stage.tile([128, 512], F32, tag="ostage",
                                         name=f"ost{qc}{dc}", bufs=2)
                    if evict_eng == "scalar":
                        nc.scalar.copy(o_stage[:], op[:])
                    else:
                        nc.vector.tensor_copy(o_stage[:], op[:])
                    nc.sync.dma_start(out[cs, dc * 512:(dc + 1) * 512],
                                      o_stage[:])
                return f

            qb_state = {}

            def emit_st(qb, p, kc):
                qs = slice(qb * QB, (qb + 1) * QB)
                ks = slice(kc * 128, (kc + 1) * 128)
                st = ps_st.tile([128, 1024], F32, tag="st", name=f"st{qb}{p}{kc}")
                mm0 = nc.tensor.matmul(st[:, 0:512], kt_sb[0:64, p, ks],
                                       qt_sb[0:64, p, qs], start=True, stop=True)
                nc.tensor.matmul(st[:, 512:1024], kt_sb[64:128, p, ks],
                                 qt_sb[64:128, p, qs], start=True, stop=True)
                e_t = ering.tile([128, 2, 512], F32R, tag="e",
                                 name=f"e{qb}{p}{kc}")
                nc.scalar.activation(
                    e_t[:], st[:].rearrange("p (h n) -> p h n", h=2),
                    EXP, scale=float(SCALE))
                return e_t, mm0

            def emit_av(qb, p, kc, avs, e_t):
                for hh in range(2):
                    nc.tensor.matmul(
                        avs[hh][0:DIM_HEAD + 1, :],
                        v_sb[:, kc, 2 * p + hh, :], e_t[:, hh, :],
                        start=(kc == 0), stop=(kc == N_KC - 1))

            def evict_pair(qb, p, avs, eng="vector"):
                den4 = qb_state[qb]["den4"]
                avsb = []
                for hh in range(2):
                    a_sb = stage.tile([DIM_HEAD + 1, 512], F32R, tag="avsb",
                                      name=f"avsb{qb}_{p}_{hh}", bufs=4)
                    if eng == "scalar":
                        nc.scalar.copy(a_sb[:], avs[hh][0:DIM_HEAD + 1, :])
                    else:
                        nc.vector.tensor_copy(a_sb[:], avs[hh][0:DIM_HEAD + 1, :])
                    k32 = 32 * (2 * p + hh)
                    nc.vector.tensor_copy(den4[k32:k32 + 1, :],
                                          avs[hh][DIM_HEAD:DIM_HEAD + 1, :])
                    avsb.append(a_sb)
                return avsb

            def finalize_pair(qb, p, avsb, order_after=None):
                den4 = qb_state[qb]["den4"]
                qs = slice(qb * QB, (qb + 1) * QB)
                rec = stage.tile([128, 512], F32, tag="rec",
                                 name=f"rec{qb}{p}", bufs=2)
                with nc.allow_low_precision(reason="softmax denom recip"):
                    nc.vector.reciprocal_approx_fast(rec[:], den4[:])
                recr = stage.tile([128, 512], F32R, tag="recr",
                                  name=f"recr{qb}{p}", bufs=2)
                nc.vector.tensor_copy(recr[:], rec[:])
                bc = ps_misc.tile([128, 512], F32, tag="mp", name=f"bc{qb}{p}")
                bcmm = nc.tensor.matmul(bc[:], pat_sb[:, p, :], recr[:],
                                        start=True, stop=True)
                if order_after is not None:
                    add_dep_helper(order_after.ins, bcmm.ins, sync=False,
                                   reason="hold bc behind ST stream")
                for hh in range(2):
                    nc.vector.tensor_mul(
                        ot_sb[hh * 64:(hh + 1) * 64, p, qs],
                        avsb[hh][0:DIM_HEAD, :],
                        bc[hh * 64:(hh + 1) * 64, :])

            def begin_qb(qb):
                den4 = stage.tile([128, 512], F32, tag="den4", name=f"den{qb}",
                                  bufs=1)
                nc.vector.memset(den4[:], 1.0)
                qb_state[qb] = dict(den4=den4)

            def new_avs(qb, p):
                return [ps_av.tile([128, 512], F32, tag=f"av{hh}",
                                   name=f"av{hh}_{qb}_{p}")
                        for hh in range(2)]

            def run_tasks(quota):
                n = 0
                while tasks and n < quota:
                    tasks.pop(0)()
                    n += 1

            # ---- startup: minimal prefix for the first ST ----
            emit_kt(0, ms=(0,))      # pair-0 rows of kt block 0
            emit_qt(0, ms=(0,))
            wv_sb = wpool.tile([128, 8, INNER], BF16)
            nc.gpsimd.dma_start(wv_sb[:],
                                wv.rearrange("(c p) m -> p c m", p=128))
            nc.sync.dma_start(v_sb[:, :, :, DIM_HEAD:DIM_HEAD + 1], vones[:])
            wo_sb = wpool.tile([128, 2, D_MODEL], BF16)
            nc.sync.dma_start(wo_sb[:], wo.rearrange("(c p) d -> p c d", p=128))
            pat_sb = wpool.tile([128, 2, 128], F32R)
            nc.sync.dma_start(pat_sb[:], pat4[:])
            run_tasks(8)             # kt0-m0 + qt0-m0 now
            emit_kt(0, ms=(1,))      # queue the rest
            emit_qt(0, ms=(1,))
            emit_vblock(0)

            # per-phase filler triggers (task pushes) and task quotas
            def phase_pushes(qb, p):
                pushes = {}
                def add(kc, fn):
                    pushes.setdefault(kc, []).append(fn)
                if qb == 0 and p == 0:
                    for n in range(1, N_QB):
                        add(4 * n - 4, lambda n=n: (emit_kt(n),
                                                    emit_vblock(n)))
                elif qb == 0 and p == 1:
                    add(0, lambda: emit_qt(1))
                else:
                    prev = qb - 1
                    if p == 0:
                        for g in range(4):
                            add(2 * g + 8,
                                lambda g=g: tasks.append(
                                    emit_outproj_chunk(prev, g)))
                    else:
                        if qb < N_QB - 1:
                            add(0, lambda: emit_qt(qb + 1))
                        for g in range(4):
                            add(2 * g + 1,
                                lambda g=g: tasks.append(
                                    emit_outproj_chunk(prev, 4 + g)))
                return pushes

            AV_LAG = 4
            phases = [(qb, p) for qb in range(N_QB) for p in range(2)]
            pending = None      # (qb, p, avs, [(kc, e_t)...])
            pending_fin = None  # (qb, p, avsb)

            for qb, p in phases:
                first_phase = (qb == 0 and p == 0)
                quota = 6 if first_phase else 1
                if p == 0:
                    begin_qb(qb)
                avs = new_avs(qb, p)
                pushes = phase_pushes(qb, p)
                eq = []
                for kc in range(N_KC):
                    for fn in pushes.get(kc, ()):
                        fn()
                    e_t, stmm = emit_st(qb, p, kc)
                    eq.append((kc, e_t))
                    if kc == AV_LAG - 1 and pending is not None:
                        pq, pp, pavs, peq = pending
                        for pkc, pe_t in peq:
                            emit_av(pq, pp, pkc, pavs, pe_t)
                        pending_fin = (pq, pp, evict_pair(pq, pp, pavs))
                        pending = None
                    if kc == 7 and pending_fin is not None:
                        fq, fp, favsb = pending_fin
                        finalize_pair(fq, fp, favsb, order_after=stmm)
                        pending_fin = None
                    if kc >= AV_LAG:
                        pkc, pe_t = eq[kc - AV_LAG]
                        emit_av(qb, p, pkc, avs, pe_t)
                    run_tasks(quota)
                pending = (qb, p, avs, eq[N_KC - AV_LAG:])

            # ---- tail: drain remaining tasks + last pair + last outproj ----
            run_tasks(len(tasks))
            pq, pp, pavs, peq = pending
            for pkc, pe_t in peq:
                emit_av(pq, pp, pkc, pavs, pe_t)
            finalize_pair(pq, pp, evict_pair(pq, pp, pavs, eng="scalar"))
            for idx in range(8):
                emit_outproj_chunk(
                    N_QB - 1, idx,
                    evict_eng=("scalar" if idx % 2 == 0 else "vector"))()
    nc.compile()
    return nc


_NC_CACHE = None


def _get_nc():
    global _NC_CACHE
    if _NC_CACHE is None:
        _NC_CACHE = build_nc()
    return _NC_CACHE


def _make_pat4():
    pat = np.zeros((128, 2, 128), np.float32)
    for p in range(2):
        for hh in range(2):
            pat[32 * (2 * p + hh), p, hh * 64:(hh + 1) * 64] = 1.0
    return pat


def make_in_maps(query, key, value, Wq, Wk, Wv, Wo):
    query = np.asarray(query, np.float32)
    key = np.asarray(key, np.float32)
    value = np.asarray(value, np.float32)
    vones = np.ones((128, N_KC, HEADS_PER_CORE, 1), np.float32)
    pat4 = _make_pat4()
    in_maps = []
    for c in range(N_CORES):
        b = c // 4
        hg = c % 4
        cols = slice(hg * INNER, (hg + 1) * INNER)
        in_maps.append({
            "xqt": np.ascontiguousarray(query[b].T).astype(NP_BF16),
            "xkt": np.ascontiguousarray(key[b].T).astype(NP_BF16),
            "xvt": np.ascontiguousarray(value[b].T).astype(NP_BF16),
            "wq": np.ascontiguousarray(Wq[:, cols]).astype(NP_BF16),
            "wk": np.ascontiguousarray(Wk[:, cols]).astype(NP_BF16),
            "wv": np.ascontiguousarray(Wv[:, cols]).astype(NP_BF16),
            "wo": np.ascontiguousarray(Wo[cols, :]).astype(NP_BF16),
            "vones": vones,
            "pat4": pat4,
        })
    return in_maps


def kernel(query, key, value, Wq, Wk, Wv, Wo, bo, _trace=False, _trace_cores=None):
    nc = _get_nc()
    in_maps = make_in_maps(query, key, value, Wq, Wk, Wv, Wo)
    res = bass_utils.run_bass_kernel_spmd(
        nc, in_maps, core_ids=list(range(N_CORES)), trace=_trace,
        trace_cores=_trace_cores)
    out = np.zeros((B, N, D_MODEL), np.float32)
    for c in range(N_CORES):
        out[c // 4] += np.asarray(res.results[c]["out"], np.float32)
    out += np.asarray(bo, np.float32)[None, None, :]
    if _trace:
        return out, res
    return out


# revision 20
# speedup vs baseline: 1.1705x; 1.0102x over previous
"""Multi-head attention (B=2, N=2048, d_model=1024, 16 heads x 64) on 8
Trainium2 NeuronCores.

Sharding: batch x head-group. Core c handles batch b = c//4 and heads
4*(c%4) .. 4*(c%4)+3. Projection weights are column-sliced (rows for Wo) so
each core computes q/k/v projections only for its 4 heads, full attention
for those heads, and a partial output projection. The host sums the four
partial outputs per batch (tensor-parallel reduce on to_out) and adds bo.

v3 structure (per core):
  - x / W inputs in bf16 (halves HBM traffic); intermediates in fp32r.
  - warmup matmuls during the initial DMA wait so the PE HAM clock gate
    is released before real work arrives.
  - ST: k^T q per head pair -> scores^T, two heads concurrently on PE
    row tiles; one ScalarE exp eviction per key chunk (the steady-state
    rate limiter at ~1.15us per 128x1024 tile).
  - AV: [v|ones]^T @ E accumulated over key chunks in fp32 PSUM.
  - All projection / output work is chopped into ~2-matmul micro-tasks
    dispatched between ST slots so the scores->exp pipeline never
    starves while PE queues stay dense.
  - softmax denominator: ones column of v, reciprocal_approx_fast,
    pattern-matmul broadcast, DVE multiply into O^T (bf16).
  - out: O^T-as-lhsT @ Wo slice -> partial [2048, 1024] fp32.
"""

import numpy as np
import ml_dtypes

import concourse.mybir as mybir
import concourse.tile as tile
from concourse import bacc
from concourse import bass_utils
from concourse.tile_rust import add_dep_helper

F32 = mybir.dt.float32
F32R = mybir.dt.float32r
BF16 = mybir.dt.bfloat16
EXP = mybir.ActivationFunctionType.Exp
NP_BF16 = ml_dtypes.bfloat16

B = 2
N = 2048
D_MODEL = 1024
NHEAD = 16
DIM_HEAD = 64
SCALE = DIM_HEAD ** (-0.5)
N_CORES = 8
HEADS_PER_CORE = 4          # 2 pairs
INNER = HEADS_PER_CORE * DIM_HEAD  # 256

QB = 512                    # query block
N_QB = N // QB              # 4
N_KC = N // 128             # 16 key chunks


def _rne11(x: np.ndarray) -> np.ndarray:
    """Round fp32 to fp32r (round-to-nearest-even, 11 mantissa bits)."""
    b = np.ascontiguousarray(x, dtype=np.float32).view(np.uint32)
    lsb = (b >> np.uint32(12)) & np.uint32(1)
    r = (b + np.uint32(0x7FF) + lsb) & np.uint32(0xFFFFF000)
    return r.view(np.float32)


def build_nc():
    nc = bacc.Bacc("TRN2", target_bir_lowering=False, debug=False,
                   num_devices=N_CORES)
    xqt = nc.dram_tensor("xqt", [D_MODEL, N], BF16, kind="ExternalInput").ap()
    xkt = nc.dram_tensor("xkt", [D_MODEL, N], BF16, kind="ExternalInput").ap()
    xvt = nc.dram_tensor("xvt", [D_MODEL, N], BF16, kind="ExternalInput").ap()
    wq = nc.dram_tensor("wq", [D_MODEL, INNER], BF16, kind="ExternalInput").ap()
    wk = nc.dram_tensor("wk", [D_MODEL, INNER], BF16, kind="ExternalInput").ap()
    wv = nc.dram_tensor("wv", [D_MODEL, INNER], BF16, kind="ExternalInput").ap()
    wo = nc.dram_tensor("wo", [INNER, D_MODEL], BF16, kind="ExternalInput").ap()
    # bc pattern: pat4[k, p, m] = 1 where head k owns output rows m in pair p
    pat4 = nc.dram_tensor("pat4", [128, 2, 128], F32R, kind="ExternalInput").ap()
    out = nc.dram_tensor("out", [N, D_MODEL], F32, kind="ExternalOutput").ap()

    with tile.TileContext(nc) as tc:
        with (
            tc.tile_pool(name="wpool", bufs=1) as wpool,
            tc.tile_pool(name="persist", bufs=1) as persist,
            tc.tile_pool(name="xk_p", bufs=2) as xk_p,
            tc.tile_pool(name="xq_p", bufs=2) as xq_p,
            tc.tile_pool(name="xv_p", bufs=2) as xv_p,
            tc.tile_pool(name="ering", bufs=9) as ering,
            tc.tile_pool(name="stage", bufs=3) as stage,
            tc.tile_pool(name="ps_st", bufs=2, space="PSUM") as ps_st,
            tc.tile_pool(name="ps_av", bufs=1, space="PSUM") as ps_av,
            tc.tile_pool(name="ps_misc", bufs=2, space="PSUM") as ps_misc,
        ):
            # ---- weight DMAs: wk/xk chain on sync (HWDGE), the rest on
            # gpsimd (SWDGE) so the scalar queue carries ONLY activations.
            wk_sb = wpool.tile([128, 8, INNER], BF16)
            nc.sync.dma_start(wk_sb[:], wk.rearrange("(c p) m -> p c m", p=128))
            wq_sb = wpool.tile([128, 8, INNER], BF16)
            nc.gpsimd.dma_start(wq_sb[:], wq.rearrange("(c p) m -> p c m", p=128))

            qt_sb = persist.tile([128, 2, N], F32R)
            kt_sb = persist.tile([128, 2, N], F32R)
            # dim-first layout: the ones row v_sb[:, 64, :, :] is one contiguous
            # 64-element block per partition -> a single legal memset
            v_sb = persist.tile([128, DIM_HEAD + 1, N_KC, HEADS_PER_CORE], F32R)
            ot_sb = persist.tile([128, 2, N], BF16)

            xqt_r = xqt.rearrange("(c p) n -> p c n", p=128)
            xkt_r = xkt.rearrange("(c p) n -> p c n", p=128)
            xvt_r = xvt.rearrange("(c p) n -> p c n", p=128)

            # ---- PE warmup: ~15 tiny matmuls during the initial DMA wait
            # release the HAM clock gate (~3.4us of activity) so real
            # projections run at 2.4 GHz from the start.
            warm_sb = stage.tile([128, 64], F32, tag="warm", name="warm",
                                 bufs=1)
            nc.vector.memset(warm_sb[:], 1.0)
            pwarm = ps_misc.tile([64, 64], F32, tag="mp", name="pwarm",
                                 padded_shape=[128, 512])
            for _ in range(15):
                nc.tensor.matmul(pwarm[:], warm_sb[:, 0:64], warm_sb[:],
                                 start=True, stop=True)

            tasks = []          # FIFO of PE micro-tasks (~<=2 big matmuls)

            xk_tiles = {}
            xq_tiles = {}

            def emit_kt(n, ms=(0, 1)):
                ns = slice(n * QB, (n + 1) * QB)
                if n not in xk_tiles:
                    xk_tiles[n] = xk_p.tile([128, 8, QB], BF16, tag="xk",
                                            name=f"xk_{n}")
                    nc.sync.dma_start(xk_tiles[n][:], xkt_r[:, :, ns])
                xk_t = xk_tiles[n]
                for m in ms:
                    st_ = {}
                    def head(m=m, st_=st_, xk_t=xk_t, n=n):
                        st_['pk'] = ps_misc.tile([128, QB], F32, tag="mp",
                                                 name=f"pk{n}{m}")
                        for c in (0, 1):
                            nc.tensor.matmul(
                                st_['pk'][:], wk_sb[:, c, m * 128:(m + 1) * 128],
                                xk_t[:, c, :], start=(c == 0), stop=False)
                    def mid(cs, m=m, st_=st_, xk_t=xk_t):
                        def f():
                            for c in cs:
                                nc.tensor.matmul(
                                    st_['pk'][:],
                                    wk_sb[:, c, m * 128:(m + 1) * 128],
                                    xk_t[:, c, :], start=False, stop=False)
                        return f
                    def fin(m=m, st_=st_, xk_t=xk_t, ns=ns):
                        for c in (6, 7):
                            nc.tensor.matmul(
                                st_['pk'][:], wk_sb[:, c, m * 128:(m + 1) * 128],
                                xk_t[:, c, :], start=False, stop=(c == 7))
                        nc.vector.tensor_copy(kt_sb[:, m, ns], st_['pk'][:])
                    tasks.extend([head, mid((2, 3)), mid((4, 5)), fin])

            def emit_qt(n, ms=(0, 1)):
                ns = slice(n * QB, (n + 1) * QB)
                if n not in xq_tiles:
                    xq_tiles[n] = xq_p.tile([128, 8, QB], BF16, tag="xq",
                                            name=f"xq_{n}")
                    nc.gpsimd.dma_start(xq_tiles[n][:], xqt_r[:, :, ns])
                xq_t = xq_tiles[n]
                for m in ms:
                    st_ = {}
                    def head(m=m, st_=st_, xq_t=xq_t, n=n):
                        st_['pq'] = ps_misc.tile([128, QB], F32, tag="mp",
                                                 name=f"pq{n}{m}")
                        for c in (0, 1):
                            nc.tensor.matmul(
                                st_['pq'][:], wq_sb[:, c, m * 128:(m + 1) * 128],
                                xq_t[:, c, :], start=(c == 0), stop=False)
                    def mid(cs, m=m, st_=st_, xq_t=xq_t):
                        def f():
                            for c in cs:
                                nc.tensor.matmul(
                                    st_['pq'][:],
                                    wq_sb[:, c, m * 128:(m + 1) * 128],
                                    xq_t[:, c, :], start=False, stop=False)
                        return f
                    def fin(m=m, st_=st_, xq_t=xq_t, ns=ns):
                        for c in (6, 7):
                            nc.tensor.matmul(
                                st_['pq'][:], wq_sb[:, c, m * 128:(m + 1) * 128],
                                xq_t[:, c, :], start=False, stop=(c == 7))
                        nc.vector.tensor_copy(qt_sb[:, m, ns], st_['pq'][:])
                    tasks.extend([head, mid((2, 3)), mid((4, 5)), fin])

            def emit_vblock(n):
                ns = slice(n * QB, (n + 1) * QB)
                xv_t = xv_p.tile([128, 8, QB], BF16, tag="xv", name=f"xv_{n}")
                nc.gpsimd.dma_start(xv_t[:], xvt_r[:, :, ns])
                for kci in range(4):
                    kc = n * 4 + kci
                    kcs = slice(kci * 128, (kci + 1) * 128)
                    st_ = {}
                    def piece(cs, first, last, kc=kc, kcs=kcs, st_=st_,
                              xv_t=xv_t):
                        def f():
                            if first:
                                st_['pva'] = ps_misc.tile(
                                    [128, INNER], F32, tag="mp",
                                    padded_shape=[128, 512], name=f"pva{kc}")
                                st_['pvb'] = ps_misc.tile(
                                    [128, INNER], F32, tag="mp",
                                    padded_shape=[128, 512], name=f"pvb{kc}")
                            for c in cs:
                                nc.tensor.matmul(
                                    st_['pva'][:], xv_t[0:64, c, kcs],
                                    wv_sb[0:64, c, :], start=(c == 0),
                                    stop=(c == 7))
                                nc.tensor.matmul(
                                    st_['pvb'][:], xv_t[64:128, c, kcs],
                                    wv_sb[64:128, c, :], start=(c == 0),
                                    stop=(c == 7))
                            if last:
                                va_sb = stage.tile([128, INNER], F32,
                                                   tag="vasb", name=f"va{kc}",
                                                   bufs=2)
                                nc.vector.tensor_copy(va_sb[:], st_['pva'][:])
                                nc.vector.scalar_tensor_tensor(
                                    v_sb[:, 0:DIM_HEAD, kc, :].rearrange(
                                        "p d h -> p h d"),
                                    st_['pvb'][:].rearrange(
                                        "p (h d) -> p h d", h=HEADS_PER_CORE),
                                    1.0,
                                    va_sb[:].rearrange(
                                        "p (h d) -> p h d", h=HEADS_PER_CORE),
                                    mybir.AluOpType.mult, mybir.AluOpType.add)
                        return f
                    tasks.append(piece((0, 1, 2, 3), True, False))
                    tasks.append(piece((4, 5, 6, 7), False, True))

            def emit_outproj_chunk(qb, idx, evict_eng="vector"):
                qc = qb * 4 + idx // 2
                dc = idx % 2
                cs = slice(qc * 128, (qc + 1) * 128)
                def f():
                    op = ps_misc.tile([128, 512], F32, tag="mp",
                                      name=f"op{qc}{dc}")
                    for ic in range(2):
                        nc.tensor.matmul(
                            op[:], ot_sb[:, ic, cs],
                            wo_sb[:, ic, dc * 512:(dc + 1) * 512],
                            start=(ic == 0), stop=(ic == 1))
                    o_stage = stage.tile([128, 512], F32, tag="ostage",
                                         name=f"ost{qc}{dc}", bufs=2)
                    if evict_eng == "scalar":
                        nc.scalar.copy(o_stage[:], op[:])
                    else:
                        nc.vector.tensor_copy(o_stage[:], op[:])
                    nc.sync.dma_start(out[cs, dc * 512:(dc + 1) * 512],
                                      o_stage[:])
                return f

            qb_state = {}

            def emit_st(qb, p, kc):
                qs = slice(qb * QB, (qb + 1) * QB)
                ks = slice(kc * 128, (kc + 1) * 128)
                st = ps_st.tile([128, 1024], F32, tag="st", name=f"st{qb}{p}{kc}")
                mm0 = nc.tensor.matmul(st[:, 0:512], kt_sb[0:64, p, ks],
                                       qt_sb[0:64, p, qs], start=True, stop=True)
                nc.tensor.matmul(st[:, 512:1024], kt_sb[64:128, p, ks],
                                 qt_sb[64:128, p, qs], start=True, stop=True)
                e_t = ering.tile([128, 2, 512], F32R, tag="e",
                                 name=f"e{qb}{p}{kc}")
                nc.scalar.activation(
                    e_t[:], st[:].rearrange("p (h n) -> p h n", h=2),
                    EXP, scale=float(SCALE))
                return e_t, mm0

            def emit_av(qb, p, kc, avs, e_t):
                for hh in range(2):
                    nc.tensor.matmul(
                        avs[hh][0:DIM_HEAD + 1, :],
                        v_sb[:, :, kc, 2 * p + hh], e_t[:, hh, :],
                        start=(kc == 0), stop=(kc == N_KC - 1))

            def evict_pair(qb, p, avs, eng="vector"):
                den4 = qb_state[qb]["den4"]
                avsb = []
                for hh in range(2):
                    a_sb = stage.tile([DIM_HEAD + 1, 512], F32R, tag="avsb",
                                      name=f"avsb{qb}_{p}_{hh}", bufs=4)
                    if eng == "scalar":
                        nc.scalar.copy(a_sb[:], avs[hh][0:DIM_HEAD + 1, :])
                    else:
                        nc.vector.tensor_copy(a_sb[:], avs[hh][0:DIM_HEAD + 1, :])
                    k32 = 32 * (2 * p + hh)
                    nc.vector.tensor_copy(den4[k32:k32 + 1, :],
                                          avs[hh][DIM_HEAD:DIM_HEAD + 1, :])
                    avsb.append(a_sb)
                return avsb

            def finalize_pair(qb, p, avsb, order_after=None):
                den4 = qb_state[qb]["den4"]
                qs = slice(qb * QB, (qb + 1) * QB)
                rec = stage.tile([128, 512], F32, tag="rec",
                                 name=f"rec{qb}{p}", bufs=2)
                with nc.allow_low_precision(reason="softmax denom recip"):
                    nc.vector.reciprocal_approx_fast(rec[:], den4[:])
                recr = stage.tile([128, 512], F32R, tag="recr",
                                  name=f"recr{qb}{p}", bufs=2)
                nc.vector.tensor_copy(recr[:], rec[:])
                bc = ps_misc.tile([128, 512], F32, tag="mp", name=f"bc{qb}{p}")
                bcmm = nc.tensor.matmul(bc[:], pat_sb[:, p, :], recr[:],
                                        start=True, stop=True)
                if order_after is not None:
                    add_dep_helper(order_after.ins, bcmm.ins, sync=False,
                                   reason="hold bc behind ST stream")
                for hh in range(2):
                    nc.vector.tensor_mul(
                        ot_sb[hh * 64:(hh + 1) * 64, p, qs],
                        avsb[hh][0:DIM_HEAD, :],
                        bc[hh * 64:(hh + 1) * 64, :])

            def begin_qb(qb):
                den4 = stage.tile([128, 512], F32, tag="den4", name=f"den{qb}",
                                  bufs=1)
                nc.vector.memset(den4[:], 1.0)
                qb_state[qb] = dict(den4=den4)

            def new_avs(qb, p):
                return [ps_av.tile([128, 512], F32, tag=f"av{hh}",
                                   name=f"av{hh}_{qb}_{p}")
                        for hh in range(2)]

            def run_tasks(quota):
                n = 0
                while tasks and n < quota:
                    tasks.pop(0)()
                    n += 1

            # ---- startup: minimal prefix for the first ST ----
            emit_kt(0, ms=(0,))      # pair-0 rows of kt block 0
            emit_qt(0, ms=(0,))
            wv_sb = wpool.tile([128, 8, INNER], BF16)
            nc.gpsimd.dma_start(wv_sb[:],
                                wv.rearrange("(c p) m -> p c m", p=128))
            # memset rejects f32r dtype; bit-identical fp32 view instead
            nc.vector.memset(v_sb[:, DIM_HEAD, :, :].bitcast(F32), 1.0)
            wo_sb = wpool.tile([128, 2, D_MODEL], BF16)
            nc.sync.dma_start(wo_sb[:], wo.rearrange("(c p) d -> p c d", p=128))
            pat_sb = wpool.tile([128, 2, 128], F32R)
            nc.sync.dma_start(pat_sb[:], pat4[:])
            run_tasks(8)             # kt0-m0 + qt0-m0 now
            emit_kt(0, ms=(1,))      # queue the rest
            emit_qt(0, ms=(1,))
            emit_vblock(0)

            # per-phase filler triggers (task pushes) and task quotas
            def phase_pushes(qb, p):
                pushes = {}
                def add(kc, fn):
                    pushes.setdefault(kc, []).append(fn)
                if qb == 0 and p == 0:
                    for n in range(1, N_QB):
                        add(4 * n - 4, lambda n=n: (emit_kt(n),
                                                    emit_vblock(n)))
                elif qb == 0 and p == 1:
                    add(0, lambda: emit_qt(1))
                else:
                    prev = qb - 1
                    if p == 0:
                        for g in range(4):
                            add(2 * g + 8,
                                lambda g=g: tasks.append(
                                    emit_outproj_chunk(prev, g)))
                    else:
                        if qb < N_QB - 1:
                            add(0, lambda: emit_qt(qb + 1))
                        for g in range(4):
                            add(2 * g + 1,
                                lambda g=g: tasks.append(
                                    emit_outproj_chunk(prev, 4 + g)))
                return pushes

            AV_LAG = 4
            phases = [(qb, p) for qb in range(N_QB) for p in range(2)]
            pending = None      # (qb, p, avs, [(kc, e_t)...])
            pending_fin = None  # (qb, p, avsb)

            for qb, p in phases:
                first_phase = (qb == 0 and p == 0)
                quota = 6 if first_phase else 1
                if p == 0:
                    begin_qb(qb)
                avs = new_avs(qb, p)
                pushes = phase_pushes(qb, p)
                eq = []
                for kc in range(N_KC):
                    for fn in pushes.get(kc, ()):
                        fn()
                    e_t, stmm = emit_st(qb, p, kc)
                    eq.append((kc, e_t))
                    if kc == AV_LAG - 1 and pending is not None:
                        pq, pp, pavs, peq = pending
                        for pkc, pe_t in peq:
                            emit_av(pq, pp, pkc, pavs, pe_t)
                        pending_fin = (pq, pp, evict_pair(pq, pp, pavs))
                        pending = None
                    if kc == 7 and pending_fin is not None:
                        fq, fp, favsb = pending_fin
                        finalize_pair(fq, fp, favsb, order_after=stmm)
                        pending_fin = None
                    if kc >= AV_LAG:
                        pkc, pe_t = eq[kc - AV_LAG]
                        emit_av(qb, p, pkc, avs, pe_t)
                    run_tasks(quota)
                pending = (qb, p, avs, eq[N_KC - AV_LAG:])

            # ---- tail: drain remaining tasks + last pair + last outproj ----
            run_tasks(len(tasks))
            pq, pp, pavs, peq = pending
            for pkc, pe_t in peq:
                emit_av(pq, pp, pkc, pavs, pe_t)
            finalize_pair(pq, pp, evict_pair(pq, pp, pavs, eng="scalar"))
            for idx in range(8):
                emit_outproj_chunk(
                    N_QB - 1, idx,
                    evict_eng=("scalar" if idx % 2 == 0 else "vector"))()
    nc.compile()
    return nc


_NC_CACHE = None


def _get_nc():
    global _NC_CACHE
    if _NC_CACHE is None:
        _NC_CACHE = build_nc()
    return _NC_CACHE


def _make_pat4():
    pat = np.zeros((128, 2, 128), np.float32)
    for p in range(2):
        for hh in range(2):
            pat[32 * (2 * p + hh), p, hh * 64:(hh + 1) * 64] = 1.0
    return pat


def make_in_maps(query, key, value, Wq, Wk, Wv, Wo):
    query = np.asarray(query, np.float32)
    key = np.asarray(key, np.float32)
    value = np.asarray(value, np.float32)
    pat4 = _make_pat4()
    in_maps = []
    for c in range(N_CORES):
        b = c // 4
        hg = c % 4
        cols = slice(hg * INNER, (hg + 1) * INNER)
        in_maps.append({
            "xqt": np.ascontiguousarray(query[b].T).astype(NP_BF16),
            "xkt": np.ascontiguousarray(key[b].T).astype(NP_BF16),
            "xvt": np.ascontiguousarray(value[b].T).astype(NP_BF16),
            "wq": np.ascontiguousarray(Wq[:, cols]).astype(NP_BF16),
            "wk": np.ascontiguousarray(Wk[:, cols]).astype(NP_BF16),
            "wv": np.ascontiguousarray(Wv[:, cols]).astype(NP_BF16),
            "wo": np.ascontiguousarray(Wo[cols, :]).astype(NP_BF16),
            "pat4": pat4,
        })
    return in_maps


def kernel(query, key, value, Wq, Wk, Wv, Wo, bo, _trace=False, _trace_cores=None):
    nc = _get_nc()
    in_maps = make_in_maps(query, key, value, Wq, Wk, Wv, Wo)
    res = bass_utils.run_bass_kernel_spmd(
        nc, in_maps, core_ids=list(range(N_CORES)), trace=_trace,
        trace_cores=_trace_cores)
    out = np.zeros((B, N, D_MODEL), np.float32)
    for c in range(N_CORES):
        out[c // 4] += np.asarray(res.results[c]["out"], np.float32)
    out += np.asarray(bo, np.float32)[None, None, :]
    if _trace:
        return out, res
    return out
